# revision 79
# baseline (speedup 1.0000x reference)
"""Trainium2 Bass kernel for windowed attention with LoRA + decomposed rel-pos bias.

Full-input contract: kernel(**inputs) takes the unsharded numpy inputs and
returns the full (64, 14, 14, 768) float32 output.

Strategy (8 NeuronCores, data-parallel over the 64-window batch, 8 windows/core):
  Host prep (numpy):
    - Fold LoRA into qkv weights:  Wq += lb_q@la_q, Wv += lb_v@la_v  (exact math).
    - Fold attention scale (2^-3, exact) into Wq / b_q; rel-pos tables get 1/scale.
    - Drop the k-projection bias entirely: (q+bq).(k+bk) differs from
      (q+bq).k by a per-query constant across keys, which softmax cancels.
    - Pre-transpose all weights + x; gather rel_pos tables with the (q-k)
      index map; replicate the one-hot key patterns across all 48 pair
      slots host-side so the device loads them with one big contiguous DMA.
  On chip (per core, all SBUF resident):
    - q/k projection into per-(parity, window*headpair) "augmented" tiles
      [128, 2, 48, 196]: 64 q/k rows + 14 rel-pos feature rows + 14 one-hot
      rows per parity so ONE matmul per key-chunk produces
      q@k^T*scale + rel_h + rel_w directly in PSUM (K-augmentation trick).
    - key chunks are 0:128 and 68:196 (overlapping) so every QK / v tile is
      a full 128 partitions — no junk PSUM rows, denominator contracts
      chunk0 over keys 0:68 only.
    - one batched exp on ScalarE per (window, head-pair): both heads' QK
      chunks share a two-bank PSUM tile; the attention loop emits one
      window ahead so TensorE never queues behind ScalarE.
    - softmax denominators of both heads accumulate side by side on PSUM
      row 0 (one matmul group), giving a single DVE reciprocal per window;
      the reciprocals bounce through DRAM for the partition broadcast and
      the in-place normalize runs on GpSimd (SBUF-only engine — it cannot
      touch PSUM, and its partition_broadcast op is broken on this path).
    - rel_h stage rides inside the k-projection window, rel_w inside the
      v-projection window (rel_w PSUM evacuation on the there-idle
      ScalarE), so the copy work hides under TensorE matmuls.
    - projection emits feature-major output (proj weights stationary, bias
      added by ScalarE per partition while evacuating PSUM); the host
      transposes back. k-projection bias is dropped entirely: softmax is
      invariant to the per-query constant (q+bq).bk.
  fp8 fast path (rel err ~1.8e-2 vs the 2e-2 gate, deterministic):
    - qaug/kaug/rel tables are fp8e4 (q' = 256*scale*q, k' = 32*k, rel rows
      at 256*(q.Rh), key patterns carry 32); exp scale 2^-14 divides out the
      8192x product scaling plus the DoubleRow pair-doubling.
    - QK matmuls run fp8 DoubleRow with a stride-0 broadcast pair (billed
      0.5 cycles/row); the k projection runs fp8 DoubleRow with REAL
      contraction pairs over (x8, Wk8) chunk pairs (quarter cost vs bf16).
    - rel matmuls are per-window fp8 DoubleRow: DR rejects nonzero dst
      partition offsets (s3d3 ISA check), so outputs land at partitions
      0:14 and the evacuation copies shift partitions into the qaug rows.
    - q/v/proj stay bf16: fp8 there costs 1.6-4% output error (gate blown).
    - q evacuations alternate ScalarE/DVE (phase 1 is otherwise
      ScalarE-bound); rel_h hx>=8 emits inside the v phase on ScalarE.
"""

import numpy as np
import ml_dtypes

B_TOTAL = 64
NCORES = 8
BPC = B_TOTAL // NCORES  # windows per core
H = W = 14
N = H * W  # 196 tokens per window
DIM = 768
NH = 12
HD = 64
DC = DIM // 128  # 6 contraction chunks
NKT0 = 128  # key chunk 0: keys 0:128
S1 = 68     # key chunk 1: keys 68:196 (full 128 rows, overlaps chunk 0)
SCALE = HD ** -0.5  # 0.125, exact power of two
NPR = BPC * (NH // 2)  # 48 (window, head-pair) slots per parity

# row maps inside the 128-partition augmented q/k tiles
# even parity (par=0): q/k rows 0:64, relh/kh-onehot 64:78, zeros 78:96,
#                      relw/kw-onehot 96:110; contraction range [0:110)
# odd  parity (par=1): relw/kw-onehot 0:14, zeros 14:32, relh/kh-onehot
#                      32:46, zeros 46:64, q/k rows 64:128; range [0:128)
K_EVEN = 110
K_ODD = 128

_NC_CACHE = {}


def build_module():
    from contextlib import ExitStack

    import concourse.tile as tile
    from concourse import bacc, mybir

    f32 = mybir.dt.float32
    bf16 = mybir.dt.bfloat16
    f8 = mybir.dt.float8e4
    AF = mybir.ActivationFunctionType
    ALU = mybir.AluOpType
    DR = mybir.MatmulPerfMode.DoubleRow

    nc = bacc.Bacc(
        "TRN2", target_bir_lowering=False, debug=False, num_devices=NCORES
    )

    T = BPC * N  # 1568 tokens per core

    xT = nc.dram_tensor("xT", [DIM, T], bf16, kind="ExternalInput").ap()
    # q/k projections produce fp8e4 tiles (scaled by S_Q/S_K host-side); the
    # QK matmul then runs in fp8 DoubleRow mode at half cost. The pair of
    # DoubleRow k-tiles both point at the same data (stride-0 broadcast), so
    # the product comes out doubled; 1/2 rides the exp scale below.
    # The k projection itself also runs in fp8 DoubleRow (x and Wk both fp8,
    # real contraction pairs), which quarters its matmul cost.
    wq = nc.dram_tensor("wq", [DIM, DIM], bf16, kind="ExternalInput").ap()
    wk8 = nc.dram_tensor("wk8", [DIM, DIM], f8, kind="ExternalInput").ap()
    xT8 = nc.dram_tensor("xT8", [DIM, T], f8, kind="ExternalInput").ap()
    wv = nc.dram_tensor("wv", [DIM, DIM], bf16, kind="ExternalInput").ap()
    pw = nc.dram_tensor("pw", [DIM, DIM], bf16, kind="ExternalInput").ap()
    bq = nc.dram_tensor("bq", [DIM], f32, kind="ExternalInput").ap()
    bv = nc.dram_tensor("bv", [DIM], bf16, kind="ExternalInput").ap()
    bp = nc.dram_tensor("bp", [DIM], f32, kind="ExternalInput").ap()
    relh = nc.dram_tensor("relh", [HD, N], f8, kind="ExternalInput").ap()
    relw = nc.dram_tensor("relw", [HD, N], f8, kind="ExternalInput").ap()
    # key patterns (value 32 = S_Q*S_K/S_R), pre-replicated across 48 slots
    oh_e = nc.dram_tensor("oh_e", [46, NPR * N], f8, kind="ExternalInput").ap()
    oh_o = nc.dram_tensor("oh_o", [64, NPR * N], f8, kind="ExternalInput").ap()
    # feature-major bf16 output; the host transposes back to token-major and
    # upcasts (bf16 rounding adds ~0.2% rel err, well within budget; halves
    # the writeback DMA time on the kernel's tail)
    out = nc.dram_tensor("out", [DIM, T], bf16, kind="ExternalOutput").ap()

    with tile.TileContext(nc) as tc, ExitStack() as ctx:
        singles = ctx.enter_context(tc.tile_pool(name="singles", bufs=1))
        attn_pool = ctx.enter_context(tc.tile_pool(name="attn", bufs=3))
        r_pool = ctx.enter_context(tc.tile_pool(name="rp", bufs=2))
        osb_pool = ctx.enter_context(tc.tile_pool(name="osb", bufs=3))
        rd_pool = ctx.enter_context(tc.tile_pool(name="rd", bufs=2, space="DRAM"))

        # ---- resident SBUF tensors; DMA order = consumption order ----
        wq_sb = singles.tile([128, DC, DIM], bf16)
        wq_r = wq.rearrange("(c p) o -> p c o", p=128)
        wk8_sb = singles.tile([128, DC, DIM], f8)
        wk8_r = wk8.rearrange("(c p) o -> p c o", p=128)
        xT_pool_cm = tc.tile_pool(name="xt", bufs=1)
        xt_pool = xT_pool_cm.__enter__()
        xT_sb = xt_pool.tile([128, DC, T], bf16)
        xT_r = xT.rearrange("(c p) t -> p c t", p=128)
        xT8_sb = xt_pool.tile([128, DC, T], f8)
        xT8_r = xT8.rearrange("(c p) t -> p c t", p=128)
        TH = T // 2
        for c in range(DC):
            # startup DMAs split across queues: xT on SP, q-weights on the
            # ScalarE HWDGE ring, k-weights + fp8 x on the GpSimd SWDGE
            # ring, so per-DMA queue/descriptor-generation overheads overlap
            nc.sync.dma_start(out=xT_sb[:, c, 0:TH], in_=xT_r[:, c, 0:TH])
            nc.scalar.dma_start(out=wq_sb[:, c, :], in_=wq_r[:, c, :])
            nc.gpsimd.dma_start(out=wk8_sb[:, c, :], in_=wk8_r[:, c, :])
        bq_sb = singles.tile([128, DC], f32)
        nc.sync.dma_start(out=bq_sb[:], in_=bq.rearrange("(c p) -> p c", p=128))
        for c in range(DC):
            nc.sync.dma_start(out=xT_sb[:, c, TH:T], in_=xT_r[:, c, TH:T])
        wv_sb = singles.tile([128, DC, DIM], bf16)
        nc.sync.dma_start(out=wv_sb[:], in_=wv.rearrange("(c p) o -> p c o", p=128))
        bv_sb = singles.tile([128, DIM], bf16)
        nc.sync.dma_start(out=bv_sb[:], in_=bv.unsqueeze(0).broadcast_to([128, DIM]))
        relh_sb = singles.tile([128, N], f8)
        nc.sync.dma_start(out=relh_sb[0:64, :], in_=relh)
        nc.sync.dma_start(out=relh_sb[64:128, :], in_=relh)
        relw_sb = singles.tile([128, N], f8)
        nc.sync.dma_start(out=relw_sb[0:64, :], in_=relw)
        nc.sync.dma_start(out=relw_sb[64:128, :], in_=relw)
        # fp8 x for the k projection: loaded after the q-phase-critical
        # tensors (k matmuls only start once the q projection finishes)
        for c in range(DC):
            nc.sync.dma_start(out=xT8_sb[:, c, :], in_=xT8_r[:, c, :])

        # par-major augmented tiles: [contraction, parity, (window,headpair), key]
        qaug = singles.tile([128, 2, NPR, N], f8)
        kaug = singles.tile([128, 2, NPR, N], f8)
        nc.sync.dma_start(
            out=kaug[64:110, 0, :, :],
            in_=oh_e.rearrange("p (s q) -> p s q", s=NPR),
        )
        nc.sync.dma_start(
            out=kaug[0:64, 1, :, :],
            in_=oh_o.rearrange("p (s q) -> p s q", s=NPR),
        )
        pw_sb = singles.tile([128, DC, DIM], bf16)
        nc.sync.dma_start(out=pw_sb[:], in_=pw.rearrange("(c p) o -> p c o", p=128))
        bp_sb = singles.tile([128, DC], f32)
        nc.sync.dma_start(out=bp_sb[:], in_=bp.rearrange("(c p) -> p c", p=128))

        # warm up the tensor engine during the startup DMA wait: the clock
        # ramp needs ~3us of sustained matmuls, so burn them on a dummy tile
        # while the first weight/activation chunks stream in
        warm_sb = singles.tile([128, 512], bf16)
        nc.vector.memset(warm_sb[:], 0.0)
        warm_cm = tc.tile_pool(name="warm", bufs=1, space="PSUM")
        warm_pool = warm_cm.__enter__()
        p_w = warm_pool.tile([128, 512], f32, tag="w")
        for _ in range(9):
            nc.tensor.matmul(
                p_w[:, 0:512],
                lhsT=warm_sb[:, 0:128],
                rhs=warm_sb[:, 0:512],
                start=True,
                stop=True,
            )
        warm_cm.__exit__(None, None, None)

        # zero the feature halves of the augmented q tiles on GpSimd
        # (SBUF-only engine, idle here); rel copies later overwrite the
        # feature rows within.
        nc.gpsimd.memset(qaug[64:128, 0, :, :], 0.0)
        nc.gpsimd.memset(qaug[0:64, 1, :, :], 0.0)

        # [token-chunk rows, window, chunk, head, hd+ones]
        # chunk 0 = tokens 0:128, chunk 1 = tokens 68:196
        vall = singles.tile([128, BPC, 2, NH, HD + 1], bf16)
        nc.gpsimd.memset(vall[:, :, :, :, HD : HD + 1], 1.0)

        # rel feature stage: 4 query-rows batched per psum tile, evacuated
        # by one DVE copy each; rel_h rides inside the phase-1 q window,
        # rel_w inside the v-projection window.
        G_GRPS = [(0, 4), (4, 4), (8, 4), (12, 2)]

        def emit_relh(hx, pool, on_act=False, grps=None, gmax=4):
            par = hx % 2
            q_rows = slice(0, 64) if par == 0 else slice(64, 128)
            relh_rows = slice(64, 78) if par == 0 else slice(32, 46)
            relh_tp = (0 if par == 0 else 64, 64 if par == 0 else 32)
            qslot = qaug[:, par, hx // 2 : NPR : 6, :]  # [128, BPC, N] view
            for g0, gl in grps or G_GRPS:
                p_r = pool.tile([128, gmax, 128], f32, tag="pr")
                for i in range(gl):
                    g = g0 + i
                    # fp8 DoubleRow per window (stride-0 pair; doubling is
                    # folded into the host rel table). DR fails the HW ISA
                    # check for nonzero dst partition offsets, so outputs
                    # land at partitions 0:14 (only the contraction-quadrant
                    # row offset is kept) and the evacuation copy shifts
                    # partitions.
                    for w in range(BPC):
                        nc.tensor.matmul(
                            p_r[0:14, i, w * W : (w + 1) * W],
                            lhsT=relh_sb[q_rows, g * W : (g + 1) * W]
                            .unsqueeze(1)
                            .broadcast_to([64, 2, W]),
                            rhs=qslot[q_rows, w, g * W : (g + 1) * W]
                            .unsqueeze(1)
                            .broadcast_to([64, 2, W]),
                            start=True,
                            stop=True,
                            tile_position=(relh_tp[0], 0),
                            perf_mode=DR,
                        )
                out_ap = qslot[relh_rows, :, :].rearrange(
                    "p b (qh qw) -> p b qh qw", qh=H
                )[:, :, g0 : g0 + gl, :]
                in_ap = p_r[0:14, 0:gl, 0 : BPC * W].rearrange(
                    "p s (b w) -> p b s w", b=BPC
                )
                if on_act:
                    # the tail of the rel_h stage rides ScalarE (slack left
                    # by the k evacuations) so DVE is drained by the time the
                    # v-projection's bias-adds need it
                    nc.scalar.activation(
                        out=out_ap, in_=in_ap, func=AF.Copy, scale=1.0
                    )
                else:
                    nc.vector.tensor_copy(out=out_ap, in_=in_ap)

        def emit_relw(hx, pool, on_act=True):
            par = hx % 2
            q_rows = slice(0, 64) if par == 0 else slice(64, 128)
            relw_rows = slice(96, 110) if par == 0 else slice(0, 14)
            relw_tp = (0 if par == 0 else 64, 96 if par == 0 else 0)
            qslot = qaug[:, par, hx // 2 : NPR : 6, :]
            for g0, gl in G_GRPS:
                p_r = pool.tile([128, 4, 128], f32, tag="pr")
                for i in range(gl):
                    g = g0 + i
                    for w in range(BPC):
                        nc.tensor.matmul(
                            p_r[0:14, i, w * W : (w + 1) * W],
                            lhsT=relw_sb[q_rows, g * W : (g + 1) * W]
                            .unsqueeze(1)
                            .broadcast_to([64, 2, W]),
                            rhs=qslot[q_rows, w, g : g + 13 * W + 1 : W]
                            .unsqueeze(1)
                            .broadcast_to([64, 2, W]),
                            start=True,
                            stop=True,
                            tile_position=(relw_tp[0], 0),
                            perf_mode=DR,
                        )
                out_ap = qslot[relw_rows, :, :].rearrange(
                    "p b (qh qw) -> p b qh qw", qh=H
                )[:, :, :, g0 : g0 + gl]
                in_ap = p_r[0:14, 0:gl, 0 : BPC * W].rearrange(
                    "p s (b q) -> p b q s", b=BPC
                )
                if on_act:
                    # ScalarE is idle during the v projection; use it here
                    # so DVE keeps its headroom for the vall bias-adds
                    nc.scalar.activation(
                        out=out_ap, in_=in_ap, func=AF.Copy, scale=1.0
                    )
                else:
                    # the last head-pair goes on DVE so attention's first
                    # exp doesn't queue behind these copies on ScalarE
                    nc.vector.tensor_copy(out=out_ap, in_=in_ap)

        # ---- phase 1: q/k projection ----
        # all three phase-1 pools open up front and close together at the
        # attention seam: no pool-transition barriers between q/k/v phases
        p2_cm = tc.tile_pool(name="p2", bufs=2, space="PSUM")
        p2_pool = p2_cm.__enter__()
        psd_cm = tc.tile_pool(name="psd", bufs=2, space="PSUM")
        psd_pool = psd_cm.__enter__()
        pq_cm = tc.tile_pool(name="pq", bufs=2, space="PSUM")
        pq_pool = pq_cm.__enter__()

        # q chunks, window-pair major (matches the DMA arrival order)
        for b2 in range(BPC // 2):
            tok = slice(2 * b2 * N, (2 * b2 + 2) * N)
            for oc in range(DC):
                p_q = pq_pool.tile([128, 512], f32, tag="pq")
                for dc in range(DC):
                    nc.tensor.matmul(
                        p_q[:, 0 : 2 * N],
                        lhsT=wq_sb[:, dc, oc * 128 : (oc + 1) * 128],
                        rhs=xT_sb[:, dc, tok],
                        start=(dc == 0),
                        stop=(dc == DC - 1),
                    )
                hh = oc  # head-pair index; heads (2*oc, 2*oc+1)
                for par in range(2):
                    rows = slice(0, 64) if par == 0 else slice(64, 128)
                    out_ap = qaug[
                        rows, par, 2 * b2 * 6 + hh : 2 * b2 * 6 + hh + 7 : 6, :
                    ]
                    in_ap = p_q[rows, 0 : 2 * N].rearrange(
                        "p (w q) -> p w q", w=2
                    )
                    if oc % 2 == par:
                        # alternate the evacuations between ScalarE and DVE:
                        # the q phase is otherwise ScalarE-bound (48 evacs +
                        # the weight-DMA issue work all queue on one engine)
                        nc.scalar.activation(
                            out=out_ap,
                            in_=in_ap,
                            func=AF.Identity,
                            bias=bq_sb[rows, oc : oc + 1],
                            scale=1.0,
                        )
                    else:
                        nc.vector.tensor_scalar_add(
                            out=out_ap,
                            in0=in_ap,
                            scalar1=bq_sb[rows, oc : oc + 1],
                        )

        # k chunks (oc-pairs share a 2-bank psum tile, batched copy, no
        # bias), with the rel_h stage interleaved: its DVE copies overlap
        # the k matmuls
        for kk in range(DC // 2):
            for b2 in range(BPC // 2):
                tok = slice(2 * b2 * N, (2 * b2 + 2) * N)
                p_2 = p2_pool.tile([128, 2, 512], f32, tag="p2")
                for s in range(2):
                    oc = 2 * kk + s
                    # fp8 DoubleRow with real contraction pairs: each matmul
                    # contracts two 128-row x-chunks at 0.5 cycles/row
                    for t in range(DC // 2):
                        nc.tensor.matmul(
                            p_2[:, s, 0 : 2 * N],
                            lhsT=wk8_sb[:, 2 * t : 2 * t + 2, oc * 128 : (oc + 1) * 128],
                            rhs=xT8_sb[:, 2 * t : 2 * t + 2, tok],
                            start=(t == 0),
                            stop=(t == DC // 2 - 1),
                            perf_mode=DR,
                        )
                for par in range(2):
                    rows = slice(0, 64) if par == 0 else slice(64, 128)
                    nc.scalar.activation(
                        out=kaug[rows, par, :, :]
                        .rearrange("p (w hh) q -> p w hh q", w=BPC)[
                            :, 2 * b2 : 2 * b2 + 2, 2 * kk : 2 * kk + 2, :
                        ],
                        in_=p_2[rows, :, 0 : 2 * N].rearrange(
                            "p s (w q) -> p w s q", w=2
                        ),
                        func=AF.Copy,
                        scale=1.0,
                    )
            # rel_h for hx 0..7 rides the k phase on DVE (the k evacuations
            # occupy ScalarE); hx 8..11 move to the v phase below, where
            # ScalarE has slack under the (PE-bound) v matmuls
            for hx in range(3 * kk, min(3 * kk + 3, 8)):
                emit_relh(hx, psd_pool)


        # ---- phase 1b: v projection, interleaved with rel_w ----
        for b in range(BPC):
            for i in range(2):  # token chunks 0:128 / 68:196
                t0 = b * N + (0 if i == 0 else S1)
                last = (b, i) == (BPC - 1, 1)
                if not last:
                    p_v = p2_pool.tile([128, 2, 512], f32, tag="p2")
                    for half in range(2):
                        for dc in range(DC):
                            nc.tensor.matmul(
                                p_v[:, half, 0:384],
                                lhsT=xT_sb[:, dc, t0 : t0 + 128],
                                rhs=wv_sb[:, dc, half * 384 : (half + 1) * 384],
                                start=(dc == 0),
                                stop=(dc == DC - 1),
                            )
                    nc.vector.tensor_tensor(
                        out=vall[:, b, i, :, 0:HD].rearrange(
                            "p (s h) d -> p s h d", s=2
                        ),
                        in0=p_v[:, :, 0:384].rearrange(
                            "p s (h d) -> p s h d", h=6
                        ),
                        in1=bv_sb[:, :].rearrange(
                            "p (s h d) -> p s h d", s=2, h=6
                        ),
                        op=ALU.add,
                    )
                else:
                    # final chunk: two 1-bank psum tiles so each half's
                    # evacuation (plain ScalarE copy) starts as soon as its
                    # own matmuls finish; the bias-add is deferred onto the
                    # (idle, SBUF-only) GpSimd engine inside the attention
                    # phase, where window 7 isn't consumed for several us.
                    # This drains the phase-1 PSUM pools ~500ns sooner; the
                    # pool-transition barrier into attention waits on them.
                    for half in range(2):
                        p_vh = p2_pool.tile([128, 512], f32, tag="p2")
                        for dc in range(DC):
                            nc.tensor.matmul(
                                p_vh[:, 0:384],
                                lhsT=xT_sb[:, dc, t0 : t0 + 128],
                                rhs=wv_sb[:, dc, half * 384 : (half + 1) * 384],
                                start=(dc == 0),
                                stop=(dc == DC - 1),
                            )
                        nc.scalar.activation(
                            out=vall[:, b, i, :, 0:HD].rearrange(
                                "p (s h) d -> p s h d", s=2
                            )[:, half],
                            in_=p_vh[:, 0:384].rearrange(
                                "p (h d) -> p h d", h=6
                            ),
                            func=AF.Copy,
                            scale=1.0,
                        )
            if b <= 3:
                emit_relh(8 + b, psd_pool, on_act=True)
            if b <= 5:
                emit_relw(2 * b, psd_pool)
                emit_relw(2 * b + 1, psd_pool)
        # deferred bias for the last chunk (see above); ordered before any
        # attention consumer of window 7 by the tile framework
        nc.gpsimd.tensor_tensor(
            out=vall[:, BPC - 1, 1, :, 0:HD],
            in0=vall[:, BPC - 1, 1, :, 0:HD],
            in1=bv_sb[:, :].rearrange("p (hh d) -> p hh d", hh=NH),
            op=ALU.add,
        )

        # xT no longer needed; free its zone for o2_all (LIFO release)
        pq_cm.__exit__(None, None, None)
        psd_cm.__exit__(None, None, None)
        p2_cm.__exit__(None, None, None)
        xT_pool_cm.__exit__(None, None, None)
        o2_pool = ctx.enter_context(tc.tile_pool(name="o2", bufs=1))
        o2_all = o2_pool.tile([128, DC, T], bf16)

        # ---- phases 2+3, head-pair major attention ----
        # one pool, three tags: same 8-bank footprint, fewer
        # pool-transition barriers on the phase seams
        pat_cm = tc.tile_pool(name="pat", bufs=2, space="PSUM")
        pat_pool = pat_cm.__enter__()
        pa_pool = po_pool = pd_pool = pat_pool

        def emit_qk(b, hx, p_a):
            par = hx % 2
            hh = hx // 2
            slot = b * 6 + hh
            KL = K_EVEN if par == 0 else K_ODD
            krange = slice(0, KL)
            # fp8 DoubleRow: both k-tiles are the same data (stride-0 pair
            # dim) so the matmul runs at half cost and yields 2x the product;
            # the exp scale divides it back out.
            q_pair = (
                qaug[krange, par, slot, :]
                .unsqueeze(1)
                .broadcast_to([KL, 2, N])
            )
            for ci, ks in enumerate((slice(0, NKT0), slice(S1, N))):
                nc.tensor.matmul(
                    p_a[:, par, ci, 0:N],
                    lhsT=kaug[krange, par, slot, ks]
                    .unsqueeze(1)
                    .broadcast_to([KL, 2, 128]),
                    rhs=q_pair,
                    start=True,
                    stop=True,
                    perf_mode=DR,
                )

        def emit_denom(b, hx, a_sb, pdf, first, last):
            # one accumulation group: row 0, even head at cols 0:N, odd head
            # at cols N:2N (start=True pre-zeroes the whole row-0 region, the
            # later matmuls accumulate onto pending-zero bytes)
            par = hx % 2
            nc.tensor.matmul(
                pdf[0:1, par * N : (par + 1) * N],
                lhsT=vall[0:S1, b, 0, hx, HD : HD + 1],
                rhs=a_sb[0:S1, par, 0, :],
                start=first,
                stop=False,
                tile_position=(0, 0),
                skip_group_check=True,
            )
            nc.tensor.matmul(
                pdf[0:1, par * N : (par + 1) * N],
                lhsT=vall[:, b, 1, hx, HD : HD + 1],
                rhs=a_sb[:, par, 1, :],
                start=False,
                stop=last,
                tile_position=(0, 0),
                skip_group_check=True,
            )

        def emit_av(b, hx, a_sb, p_o):
            par = hx % 2
            rows = slice(0, 64) if par == 0 else slice(64, 128)
            av_tp = (0, 0) if par == 0 else (0, 64)
            nc.tensor.matmul(
                p_o[rows, 0:N],
                lhsT=vall[0:S1, b, 0, hx, 0:HD],
                rhs=a_sb[0:S1, par, 0, :],
                start=True,
                stop=False,
                tile_position=av_tp,
                skip_group_check=True,
            )
            nc.tensor.matmul(
                p_o[rows, 0:N],
                lhsT=vall[:, b, 1, hx, 0:HD],
                rhs=a_sb[:, par, 1, :],
                start=False,
                stop=True,
                tile_position=av_tp,
                skip_group_check=True,
            )

        for hh in range(NH // 2):
            h0, h1 = 2 * hh, 2 * hh + 1
            r_hh = r_pool.tile([1, BPC, 2, N], bf16, tag="rw")
            rb_hh = r_pool.tile([128, BPC, N], bf16, tag="rb")

            def flush(b, a_sb, r_hh=r_hh, rb_hh=rb_hh, hh=hh, h0=h0, h1=h1):
                """Post-exp work for window b: denoms, AV, recip, bcast, mult."""
                p_dd = pd_pool.tile([128, 4, 128], f32, tag="pd")
                pdf = p_dd.rearrange("p s c -> p (s c)")
                emit_denom(b, h0, a_sb, pdf, True, False)
                emit_denom(b, h1, a_sb, pdf, False, True)
                p_o = po_pool.tile([128, 512], f32, tag="po")
                emit_av(b, h0, a_sb, p_o)
                emit_av(b, h1, a_sb, p_o)
                with nc.allow_low_precision(reason="bf16 softmax recip"):
                    nc.vector.reciprocal(
                        out=r_hh[0:1, b, :, :], in_=pdf[0:1, 0 : 2 * N]
                    )
                # evacuate PSUM unnormalized; the batched normalize happens
                # after the per-head broadcast bounce below
                nc.vector.tensor_copy(
                    out=o2_all[:, hh, b * N : (b + 1) * N],
                    in_=p_o[:, 0:N],
                )

            def bounce(half, r_hh=r_hh, rb_hh=rb_hh, hh=hh):
                """Partition-broadcast the reciprocals of 4 windows with a
                DRAM bounce, then normalize o2 in place on the (otherwise
                idle, SBUF-only) GpSimd engine."""
                HB = BPC // 2
                bs = slice(half * HB, (half + 1) * HB)
                dd = rd_pool.tile([2, HB, N], bf16, tag="rd")
                nc.sync.dma_start(out=dd[0:1, :, :], in_=r_hh[0:1, bs, 0, :])
                nc.sync.dma_start(out=dd[1:2, :, :], in_=r_hh[0:1, bs, 1, :])
                nc.sync.dma_start(
                    out=rb_hh[0:64, bs, :],
                    in_=dd[0:1, :, :].broadcast_to([64, HB, N]),
                )
                nc.sync.dma_start(
                    out=rb_hh[64:128, bs, :],
                    in_=dd[1:2, :, :].broadcast_to([64, HB, N]),
                )
                o2v = o2_all[:, hh, :].rearrange("p (b q) -> p b q", b=BPC)
                nc.gpsimd.tensor_tensor(
                    out=o2v[:, bs, :],
                    in0=o2v[:, bs, :],
                    in1=rb_hh[:, bs, :],
                    op=ALU.mult,
                )

            pend = None
            for b in range(BPC):
                # both parities of the head-pair share one 2-bank psum tile
                # so a single exp instruction covers all four QK chunks;
                # emission runs one window ahead of the post-exp work so the
                # PE queue never stalls behind ScalarE.
                p_a = pa_pool.tile([128, 2, 2, 256], f32, tag="pa")
                a_sb = attn_pool.tile([128, 2, 2, N], bf16, tag="a")
                emit_qk(b, h0, p_a)
                emit_qk(b, h1, p_a)
                nc.scalar.activation(
                    out=a_sb[:],
                    in_=p_a[:, :, :, 0:N],
                    func=AF.Exp,
                    # 1/(S_Q*S_K) = 2^-13, and another 1/2 for the DoubleRow
                    # pair-doubling of the QK products
                    scale=1.0 / 16384.0,
                )
                if pend is not None:
                    flush(*pend)
                pend = (b, a_sb)
                if b == BPC // 2:
                    bounce(0)
            flush(*pend)
            bounce(1)

        pat_cm.__exit__(None, None, None)

        # ---- phase 4: projection, feature-major output ----
        # lhsT = proj weights (stationary), rhs = o2 token stream; the bias
        # is per-partition here so ScalarE adds it while evacuating PSUM.
        # Two token-half sweeps: sweep 0 (tokens 0:1024) depends only on the
        # early softmax normalizes, so it never stalls on the final
        # head-pair's bounce; the very last writeback rides the ScalarE DMA
        # ring so it issues immediately after its own evacuation.
        pp_cm = tc.tile_pool(name="pp", bufs=6, space="PSUM")
        pp_pool = pp_cm.__enter__()
        SWEEPS = [
            (0, 784, [(0, 512), (512, 272)]),
            (784, 784, [(784, 512), (1296, 272)]),
        ]
        for si, (base, blen, chunks) in enumerate(SWEEPS):
            for fc in range(DC):
                final_fc = si == 1 and fc == DC - 1
                o_sb = osb_pool.tile([128, blen], bf16, tag=f"osb{si}")
                for ci, (t0, tn) in enumerate(chunks):
                    p_p = pp_pool.tile([128, 512], f32, tag="pp")
                    for cc in range(DC):
                        nc.tensor.matmul(
                            p_p[:, 0:tn],
                            lhsT=pw_sb[:, cc, fc * 128 : (fc + 1) * 128],
                            rhs=o2_all[:, cc, t0 : t0 + tn],
                            start=(cc == 0),
                            stop=(cc == DC - 1),
                        )
                    nc.scalar.activation(
                        out=o_sb[:, t0 - base : t0 - base + tn],
                        in_=p_p[:, 0:tn],
                        func=AF.Identity,
                        bias=bp_sb[:, fc : fc + 1],
                        scale=1.0,
                    )
                    if final_fc:
                        # per-chunk DMAs: the 512-token chunk streams out on
                        # the SP ring while the final 272-token chunk
                        # computes; the last chunk rides the ScalarE ring so
                        # the two DGE pipelines overlap on the kernel tail
                        eng = nc.scalar if ci == 0 else nc.sync
                        eng.dma_start(
                            out=out[fc * 128 : (fc + 1) * 128, t0 : t0 + tn],
                            in_=o_sb[:, t0 - base : t0 - base + tn],
                        )
                if not final_fc:
                    nc.sync.dma_start(
                        out=out[fc * 128 : (fc + 1) * 128, base : base + blen],
                        in_=o_sb[:],
                    )
        pp_cm.__exit__(None, None, None)

    nc.finalize()
    return nc


def _host_prep(inputs):
    bf16 = ml_dtypes.bfloat16
    e4m3 = ml_dtypes.float8_e4m3
    x = np.asarray(inputs["x"], np.float32)
    qkv_w = np.asarray(inputs["qkv_w"], np.float32)
    qkv_b = np.asarray(inputs["qkv_b"], np.float32)
    proj_w = np.asarray(inputs["proj_w"], np.float32)
    proj_b = np.asarray(inputs["proj_b"], np.float32)
    la_q = np.asarray(inputs["la_q"], np.float32)
    lb_q = np.asarray(inputs["lb_q"], np.float32)
    la_v = np.asarray(inputs["la_v"], np.float32)
    lb_v = np.asarray(inputs["lb_v"], np.float32)
    rel_pos_h = np.asarray(inputs["rel_pos_h"], np.float32)
    rel_pos_w = np.asarray(inputs["rel_pos_w"], np.float32)

    Wq = qkv_w[:DIM] + lb_q @ la_q
    Wk = qkv_w[DIM : 2 * DIM]
    Wv = qkv_w[2 * DIM :] + lb_v @ la_v

    # q tile holds S_Q*scale*q, k tile S_K*k (fp8e4 ranges ~<130 each);
    # the exp scale divides out S_Q*S_K (and the DoubleRow pair-doubling)
    S_Q, S_K = 256.0, 32.0
    wq_host = np.ascontiguousarray((S_Q * SCALE * Wq).T.astype(bf16))
    wk8_host = np.ascontiguousarray((S_K * Wk).T.astype(e4m3))
    wv_host = np.ascontiguousarray(Wv.T.astype(bf16))
    pw_host = np.ascontiguousarray(proj_w.T.astype(bf16))
    bq_host = (S_Q * SCALE * qkv_b[:DIM]).astype(np.float32)
    bv_host = np.ascontiguousarray(qkv_b[2 * DIM :].astype(bf16))
    bp_host = proj_b.astype(np.float32)

    idx = np.arange(H)[:, None] - np.arange(H)[None, :] + (H - 1)
    Rh = rel_pos_h[idx]  # [qh, kh_j, hd]
    Rw = rel_pos_w[idx]  # [qw, kw_j, hd]
    # rel matmuls read fp8 q' = S_Q*scale*q, so (Rh/scale)*q' lands the
    # feature rows at S_Q*(q.Rh); the key patterns below carry
    # S_Q*S_K/S_Q = S_K = 32 to match the q.k product scale.
    # extra 1/2: the rel matmuls run in DoubleRow pair mode (pair doubling)
    relh_host = np.ascontiguousarray(
        (Rh / SCALE / 2).transpose(2, 0, 1).reshape(HD, N).astype(e4m3)
    )
    relw_host = np.ascontiguousarray(
        (Rw / SCALE / 2).transpose(2, 0, 1).reshape(HD, N).astype(e4m3)
    )

    kt = np.arange(N)
    oh_kh = 32.0 * (kt[None, :] // W == np.arange(H)[:, None]).astype(
        np.float32
    )
    oh_kw = 32.0 * (kt[None, :] % W == np.arange(W)[:, None]).astype(
        np.float32
    )
    oh_kh = oh_kh.astype(e4m3)
    oh_kw = oh_kw.astype(e4m3)
    z18 = np.zeros((18, N), e4m3)
    oh_e_1 = np.concatenate([oh_kh, z18, oh_kw], 0)  # [46, 196]
    oh_o_1 = np.concatenate([oh_kw, z18, oh_kh, z18], 0)  # [64, 196]
    # replicate across the 48 (window, head-pair) slots -> contiguous DMA
    oh_e_host = np.ascontiguousarray(
        np.broadcast_to(oh_e_1[:, None, :], (46, NPR, N)).reshape(46, NPR * N)
    )
    oh_o_host = np.ascontiguousarray(
        np.broadcast_to(oh_o_1[:, None, :], (64, NPR, N)).reshape(64, NPR * N)
    )

    shared = {
        "wq": wq_host,
        "wk8": wk8_host,
        "wv": wv_host,
        "pw": pw_host,
        "bq": bq_host,
        "bv": bv_host,
        "bp": bp_host,
        "relh": relh_host,
        "relw": relw_host,
        "oh_e": oh_e_host,
        "oh_o": oh_o_host,
    }

    x_flat = x.reshape(B_TOTAL, N, DIM)
    in_maps = []
    for c in range(NCORES):
        xc = x_flat[c * BPC : (c + 1) * BPC].reshape(BPC * N, DIM)
        xT_c = np.ascontiguousarray(xc.T.astype(bf16))
        m = dict(shared)
        m["xT"] = xT_c
        m["xT8"] = np.ascontiguousarray(xc.T.astype(e4m3))
        in_maps.append(m)
    return in_maps


def kernel(**inputs):
    from concourse import bass_utils

    if "nc" not in _NC_CACHE:
        _NC_CACHE["nc"] = build_module()
    nc = _NC_CACHE["nc"]
    in_maps = _host_prep(inputs)
    res = bass_utils.run_bass_kernel_spmd(
        nc, in_maps, core_ids=list(range(NCORES))
    )
    outs = [
        np.ascontiguousarray(r["out"].astype(np.float32).T).reshape(
            BPC, H, W, DIM
        )
        for r in res.results
    ]
    return np.concatenate(outs, 0)

